# revision 6
# baseline (speedup 1.0000x reference)
"""Multi-head attention (B=4, L=1024, D=1024, H=16) on 8 TRN2 NeuronCores.

v3: head-split data-parallel sharding. Core c handles batch c//2 and HEADS
[8*(c%2), 8*(c%2)+8) over ALL 1024 queries (v2 split queries instead and
computed K/V projections redundantly on both cores of a batch pair). The
out-projection is computed as a PARTIAL sum over this core's 512 vd dims;
the host adds the two partials of each batch pair. Per-core matmul work
drops 8.59 -> 6.44 GFLOP (the ideal 1/8 of total).

Input DMA: all tensors are pre-swizzled on the host into exact SBUF images
[128, N] so every DMA descriptor is a multi-KB contiguous per-partition
line (v2's "(t p) n -> p t n" views produced 1-2KB descriptors and landed
at ~200 GB/s; Wv arrived at 48us and stalled the whole pipeline).

Layouts per core (hh = c%2 head half, hs = 512*hh slice of VD):
  inA [128, 12288] = qT (8 kd-blocks x 1024 q) | Wq[:, hs] (8 x 512)
  inB [128, 12288] = xT (8 kd-blocks x 1024 k) | Wk[:, hs] (8 x 512)
  inC [128, 4096]  = Wv[:, hs]  (8 kd-blocks x 512)
  inD [128, 4096]  = Wo[hs, :]  (4 j-blocks x 1024)
  consts [128, 24] = bq(4) | bk(4) | kbias(8) | q_mask(8)
  out [128, 8192]  = 16 tiles (qt, n) of [128 q, 512 d] partial O-proj

Compute (all transposed, no transposes anywhere):
  Q^T[vd, q] = Wq(lhsT) @ qT  (+bq per-partition)   4 j-blocks x 2 q-halves
  K^T[vd, k] = Wk(lhsT) @ xT  (+bk)                 4 j-blocks x 2 k-halves
  V  [k, vd] = xT(lhsT) @ Wv                        8 k-blocks
  Units u=0..7 = (qh, j): S^T[k,2,q] per head pair (row-tiled K=64 pair),
  es = exp(S/8 + kbias) (ScalarE), acc = sum_kt es (DVE),
  den = reduce_C(acc) on GPSIMD, 1/den via DVE approx_fast,
  srs = partition_broadcast(1/den) on GPSIMD (per head half),
  O^T = [V_hA|V_hB](col-tiled M=64) @ es, oTs = O^T * srs (DVE).
  out tile (qt,n) = (sum_j oTs[j][:,qr] @ Wo_j) * q_mask  (partial)
"""

import os

os.environ.setdefault("MYCRO_LOCAL_CACHE", "1")

import numpy as np
import ml_dtypes

BF16 = ml_dtypes.bfloat16

B, LQ, LK = 4, 1024, 1024
D = 1024
H, DH = 16, 64
HC = 8          # heads per core
VS = 512        # vd dims per core
QS = 512        # queries per attention unit (2 units of 512 = 1024)
NCORES = 8
NEG = -1e4

_NC_CACHE = {}


def _build_nc():
    import concourse.bacc as bacc
    import concourse.mybir as mybir
    import concourse.tile as tile

    dt = mybir.dt

    nc = bacc.Bacc(
        "TRN2",
        debug=False,
        target_bir_lowering=False,
        num_devices=NCORES,
    )

    def din(name, shape, dtype):
        return nc.dram_tensor(name, shape, dtype, kind="ExternalInput").ap()

    aps = {
        "inA": din("inA", [128, 12288], dt.bfloat16),
        "inB": din("inB", [128, 12288], dt.bfloat16),
        "inC": din("inC", [128, 4096], dt.bfloat16),
        "inD": din("inD", [128, 4096], dt.bfloat16),
        "consts": din("consts", [128, 24], dt.float32),
        "out": nc.dram_tensor("out", [128, 8192], dt.bfloat16,
                              kind="ExternalOutput").ap(),
    }

    with tile.TileContext(nc) as tc:
        _body(tc, dt, mybir, aps)

    nc.compile()
    return nc


def _body(tc, dt, mybir, aps):
    from contextlib import ExitStack
    from concourse.tile import add_dep_helper

    ALU = mybir.AluOpType
    AF = mybir.ActivationFunctionType
    AX = mybir.AxisListType
    import concourse.bass_isa as bass_isa
    nc = tc.nc
    with ExitStack() as ctx:
        const = ctx.enter_context(tc.tile_pool(name="const", bufs=1))
        espool = ctx.enter_context(tc.tile_pool(name="es", bufs=12))
        accpool = ctx.enter_context(tc.tile_pool(name="acc", bufs=3))
        scpool = ctx.enter_context(tc.tile_pool(name="sc", bufs=2))
        srspool = ctx.enter_context(tc.tile_pool(name="srs", bufs=2))
        spair = ctx.enter_context(tc.tile_pool(name="sp", bufs=2, space="PSUM"))
        oppool = ctx.enter_context(tc.tile_pool(name="op", bufs=2, space="PSUM"))
        gpool = ctx.enter_context(tc.tile_pool(name="g", bufs=2, space="PSUM"))
        opool = ctx.enter_context(tc.tile_pool(name="osb", bufs=3))

        def ctile(shape, dtype, tag):
            return const.tile(shape, dtype, tag=tag, name=tag)

        def gtile():
            return gpool.tile([128, 512], dt.float32, tag="g", name="g")

        # ---- consts ----
        cst = ctile([128, 24], dt.float32, "cst")
        nc.sync.dma_start(cst[:], aps["consts"][:, :])

        bq_c = lambda j: cst[:, j:j + 1]
        bk_c = lambda j: cst[:, 4 + j:5 + j]
        kb_c = lambda kt: cst[:, 8 + kt:9 + kt]
        qm_c = lambda qt: cst[:, 16 + qt:17 + qt]

        ones1 = ctile([1, 128], dt.bfloat16, "ones1")
        nc.vector.memset(ones1[:], 1.0)
        ones512 = ctile([1, 512], dt.bfloat16, "ones512")
        nc.vector.memset(ones512[:], 1.0)
        ones64 = ctile([1, 64], dt.bfloat16, "ones64")
        nc.vector.memset(ones64[:], 1.0)
        onescol = ctile([128, 1], dt.bfloat16, "onescol")
        nc.vector.memset(onescol[:], 1.0)
        ejunk = ctile([1, 16], dt.float32, "ejunk")
        nc.vector.memset(ejunk[:], 1.0)
        # pull the exp ACT table load off the critical path
        nc.scalar.activation(ejunk[:], ejunk[:], AF.Exp, bias=0.0, scale=1.0)

        # ---- keep-alive matmuls (bridge consts->inA landing, warm HAM) ----
        ka = gtile()
        for _ in range(10):
            nc.tensor.matmul(ka[:], ones1[:], ones512[:], start=True, stop=True)

        # ---- input loads, phase-serialized A -> B -> C -> D via direct
        # DMA->DMA deps. Each dma_start moves a contiguous [128, N] SBUF
        # image (24KB/partition descriptors). ----
        inA_sb = ctile([128, 12288], dt.bfloat16, "inA")
        inB_sb = ctile([128, 12288], dt.bfloat16, "inB")
        wv_sb = ctile([128, 4096], dt.bfloat16, "wv")
        wo_sb = ctile([128, 4096], dt.bfloat16, "wo")

        engs = [nc.sync, nc.scalar]
        ai = []
        for i, eng in ((0, nc.sync), (1, nc.scalar), (2, nc.sync)):
            c = slice(4096 * i, 4096 * (i + 1))
            ai.append(eng.dma_start(inA_sb[:, c], aps["inA"][:, c]))
        bi = []
        for i, eng in ((0, nc.sync), (1, nc.scalar), (2, nc.sync)):
            c = slice(4096 * i, 4096 * (i + 1))
            bi.append(eng.dma_start(inB_sb[:, c], aps["inB"][:, c]))
        ci = [nc.sync.dma_start(wv_sb[:], aps["inC"][:, :])]
        di = [nc.sync.dma_start(wo_sb[:], aps["inD"][:, :])]
        for nxt, prv in ((bi, ai), (ci, bi), (di, ci)):
            for n_ in nxt:
                for p_ in prv:
                    add_dep_helper(n_.ins, p_.ins, reason="dma phase order")

        # views into the flat input tiles
        def qT_v(kd, qh):
            return inA_sb[:, 1024 * kd + 512 * qh:1024 * kd + 512 * (qh + 1)]

        def wq_v(kd, j):
            return inA_sb[:, 8192 + 512 * kd + 128 * j:
                          8192 + 512 * kd + 128 * (j + 1)]

        def xT_v(kd, n):
            return inB_sb[:, 1024 * kd + 512 * n:1024 * kd + 512 * (n + 1)]

        def xT_vb(kd, t):
            return inB_sb[:, 1024 * kd + 128 * t:1024 * kd + 128 * (t + 1)]

        def wk_v(kd, j):
            return inB_sb[:, 8192 + 512 * kd + 128 * j:
                          8192 + 512 * kd + 128 * (j + 1)]

        def wv_v(kd):
            return wv_sb[:, 512 * kd:512 * (kd + 1)]

        def wo_v(j, n):
            return wo_sb[:, 1024 * j + 512 * n:1024 * j + 512 * (n + 1)]

        # ---- projections ----
        qTp = [ctile([128, 1024], dt.bfloat16, f"qTp{j}") for j in range(4)]
        kT_sb = [ctile([128, 1024], dt.bfloat16, f"kT{j}") for j in range(4)]
        v_sb = [ctile([128, 512], dt.bfloat16, f"v{t}") for t in range(8)]

        def q_proj(j, qh):
            c = slice(512 * qh, 512 * (qh + 1))
            ps = gtile()
            for kd in range(8):
                nc.tensor.matmul(ps[:], wq_v(kd, j), qT_v(kd, qh),
                                 start=(kd == 0), stop=(kd == 7))
            nc.vector.tensor_scalar_add(qTp[j][:, c], ps[:], bq_c(j))

        def k_proj(j, n):
            c = slice(512 * n, 512 * (n + 1))
            ps = gtile()
            for kd in range(8):
                nc.tensor.matmul(ps[:], wk_v(kd, j), xT_v(kd, n),
                                 start=(kd == 0), stop=(kd == 7))
            nc.vector.tensor_scalar_add(kT_sb[j][:, c], ps[:], bk_c(j))

        def v_proj(t):
            ps = gtile()
            for kd in range(8):
                nc.tensor.matmul(ps[:], xT_vb(kd, t), wv_v(kd),
                                 start=(kd == 0), stop=(kd == 7))
            nc.vector.tensor_copy(v_sb[t][:], ps[:])

        # ---- attention units: u -> (qh, j) ----
        UNITS = [(0, 0), (0, 1), (0, 2), (0, 3), (1, 0), (1, 1), (1, 2), (1, 3)]
        oTs = [ctile([128, 1024], dt.bfloat16, f"oTs{j}") for j in range(4)]
        es_tiles = {}
        acc_last = {}
        srs_of = {}

        def s_stage(u, kt):
            qh, j = UNITS[u]
            qs = slice(512 * qh, 512 * (qh + 1))
            kc = slice(128 * kt, 128 * (kt + 1))
            sp = spair.tile([128, 2, 512], dt.float32, tag="sp", name="sp")
            nc.tensor.matmul(sp[:, 0, :], kT_sb[j][0:64, kc],
                             qTp[j][0:64, qs], start=True, stop=True)
            nc.tensor.matmul(sp[:, 1, :], kT_sb[j][64:128, kc],
                             qTp[j][64:128, qs], start=True, stop=True)
            es = espool.tile([128, 2, 512], dt.bfloat16, tag="es", name="es")
            nc.scalar.activation(es[:], sp[:], AF.Exp,
                                 bias=kb_c(kt), scale=0.125)
            es_tiles[(u, kt)] = es
            if kt == 0:
                acc_last[u] = es
            else:
                a = accpool.tile([128, 2, 512], dt.bfloat16, tag="acc",
                                 name="acc")
                prev = acc_last[u]
                nc.vector.tensor_add(
                    a[:].rearrange("p h q -> p (h q)"),
                    prev[:].rearrange("p h q -> p (h q)"),
                    es[:].rearrange("p h q -> p (h q)"))
                acc_last[u] = a

        def o_stage(u, kt, oP):
            qh, j = UNITS[u]
            es = es_tiles.pop((u, kt))
            nc.tensor.matmul(oP[0:64, :], v_sb[kt][:, 128 * j:128 * j + 64],
                             es[:, 0, :], start=(kt == 0), stop=(kt == 7),
                             tile_position=(0, 0), skip_group_check=True)
            nc.tensor.matmul(oP[64:128, :],
                             v_sb[kt][:, 128 * j + 64:128 * j + 128],
                             es[:, 1, :], start=(kt == 0), stop=(kt == 7),
                             tile_position=(0, 64), skip_group_check=True)

        def den_stage(u):
            # den = sum over keys (partitions) of acc, via GPSIMD C-reduce;
            # reciprocal on DVE (approx_fast needs partition-0 input - den
            # lands on partition 0 by construction).
            a = acc_last.pop(u)
            den = scpool.tile([128, 2, 512], dt.float32, tag="den", name="den")
            nc.gpsimd.partition_all_reduce(den[:], a[:], channels=128,
                                           reduce_op=bass_isa.ReduceOp.add)
            sca = scpool.tile([1, 1024], dt.float32, tag="sca", name="sca")
            nc.vector.reciprocal_approx_fast(
                out=sca[:], in_=den[0:1, :, :].rearrange("p h q -> p (h q)"))
            # HW quirk: partition_broadcast to an out range based at
            # partition 64 writes garbage (verified in isolation); broadcast
            # each head's 1/den to ALL 128 partitions instead.
            srsA = srspool.tile([128, 512], dt.float32, tag="srsA", name="srsA")
            nc.gpsimd.partition_broadcast(srsA[:, :], sca[0:1, 0:512])
            srsB = srspool.tile([128, 512], dt.float32, tag="srsB", name="srsB")
            nc.gpsimd.partition_broadcast(srsB[:, :], sca[0:1, 512:1024])
            srs_of[u] = (srsA, srsB)

        def sr_stage(u, oP):
            qh, j = UNITS[u]
            qs = slice(512 * qh, 512 * (qh + 1))
            srsA, srsB = srs_of.pop(u)
            nc.vector.tensor_mul(oTs[j][0:64, qs], oP[0:64, :], srsA[0:64, :])
            nc.vector.tensor_mul(oTs[j][64:128, qs], oP[64:128, :],
                                 srsB[64:128, :])

        # ---- out-projection (partial over this core's 512 vd dims) ----
        dei = [0]

        def out_dma(ot, qt, n):
            c = slice(512 * (2 * qt + n), 512 * (2 * qt + n + 1))
            nc.sync.dma_start(aps["out"][:, c], ot[:])
            dei[0] += 1

        def out_tile(qt, n):
            c = slice(512 * n, 512 * (n + 1))
            qr = slice(128 * qt, 128 * (qt + 1))
            ps = gtile()
            for j in range(4):
                nc.tensor.matmul(ps[:], oTs[j][:, qr], wo_v(j, n),
                                 start=(j == 0), stop=(j == 3))
            ot = opool.tile([128, 512], dt.bfloat16, tag="osb", name="osb")
            nc.vector.tensor_scalar_mul(ot[:], ps[:], qm_c(qt))
            out_dma(ot, qt, n)

        stage_sbuf = {}

        def stage_partial(qt, n, upto):
            c = slice(512 * n, 512 * (n + 1))
            qr = slice(128 * qt, 128 * (qt + 1))
            ps = gtile()
            for j in range(upto):
                nc.tensor.matmul(ps[:], oTs[j][:, qr], wo_v(j, n),
                                 start=(j == 0), stop=(j == upto - 1))
            st = ctile([128, 512], dt.float32, f"stg{qt}{n}")
            nc.vector.tensor_scalar_mul(st[:], ps[:], qm_c(qt))
            stage_sbuf[(qt, n)] = (st, upto)

        def drain_mm(qt, n, ps, j, upto):
            c = slice(512 * n, 512 * (n + 1))
            qr = slice(128 * qt, 128 * (qt + 1))
            nc.tensor.matmul(ps[:], oTs[j][:, qr], wo_v(j, n),
                             start=(j == upto), stop=(j == 3))

        def drain_fin(qt, n, ps):
            st, _ = stage_sbuf[(qt, n)]
            ot = opool.tile([128, 512], dt.bfloat16, tag="osb", name="osb")
            nc.vector.scalar_tensor_tensor(
                ot[:], ps[:], qm_c(qt), st[:], op0=ALU.mult, op1=ALU.add)
            out_dma(ot, qt, n)

        # ---- schedule ----
        LOOK = 2
        s_cursor = [0]

        def advance_s(upto):
            while s_cursor[0] < min(upto, 64):
                uu, kk = divmod(s_cursor[0], 8)
                s_stage(uu, kk)
                s_cursor[0] += 1

        # per-(unit, kt2-block) PE filler emissions. Placement rules (the PE
        # queue is in-order, advance_s at block (u,6) emits next-unit S):
        # k(j',0) before block (u_{j'-1}, 6); k(j',1) before block (u_{j'}, 2);
        # v(t) before block (u0, t).
        fillers = {
            (0, 0): [lambda: v_proj(2), lambda: v_proj(3)],
            (0, 2): [lambda: v_proj(4), lambda: v_proj(5)],
            (0, 4): [lambda: k_proj(1, 0), lambda: v_proj(6), lambda: v_proj(7)],
            (0, 6): [lambda: k_proj(1, 1)],
            (1, 0): [lambda: k_proj(2, 0)],
            (1, 2): [lambda: q_proj(0, 1)],
            (1, 4): [lambda: k_proj(2, 1)],
            (2, 0): [lambda: k_proj(3, 0)],
            (2, 2): [lambda: q_proj(1, 1)],
            (2, 4): [lambda: k_proj(3, 1)],
            (3, 0): [lambda: q_proj(2, 1)],
            (3, 4): [lambda: q_proj(3, 1)],
            # qh=0 out tiles once sr(u3) has run (block (4,2))
            (4, 4): [lambda: out_tile(0, 0), lambda: out_tile(0, 1)],
            (4, 6): [lambda: out_tile(1, 0), lambda: out_tile(1, 1)],
            (5, 0): [lambda: out_tile(2, 0), lambda: out_tile(2, 1)],
            (5, 2): [lambda: out_tile(3, 0), lambda: out_tile(3, 1)],
            # qh=1 partial staging: j0,j1 after sr(u5) at (6,2); j0..j2
            # after sr(u6) at (7,2)
            (6, 4): [lambda: stage_partial(4, 0, 2),
                     lambda: stage_partial(4, 1, 2)],
            (6, 6): [lambda: stage_partial(5, 0, 2),
                     lambda: stage_partial(5, 1, 2)],
            (7, 2): [lambda: stage_partial(6, 0, 3),
                     lambda: stage_partial(6, 1, 3)],
            (7, 4): [lambda: stage_partial(7, 0, 3),
                     lambda: stage_partial(7, 1, 3)],
        }

        # prologue
        q_proj(0, 0)
        q_proj(1, 0)
        q_proj(2, 0)
        q_proj(3, 0)
        k_proj(0, 0)
        k_proj(0, 1)
        advance_s(2)
        v_proj(0)
        v_proj(1)

        oP_prev = None
        for u in range(8):
            oP = oppool.tile([128, 512], dt.float32, tag="op", name="op")
            for kt2 in range(0, 8, 2):
                if kt2 == 2 and u > 0:
                    sr_stage(u - 1, oP_prev)
                advance_s(8 * u + kt2 + 2 + LOOK)
                o_stage(u, kt2, oP)
                o_stage(u, kt2 + 1, oP)
                for f in fillers.get((u, kt2), []):
                    f()
                for f in fillers.get((u, kt2 + 1), []):
                    f()
            den_stage(u)
            oP_prev = oP

        # ---- drain ----
        # cover the den(7)->recip->broadcast window with drain matmuls that
        # only need oTs[0..2] (qh=1), then scale unit 7 and finish.
        psA = gtile()
        drain_mm(4, 0, psA, 2, 2)
        psB = gtile()
        drain_mm(4, 1, psB, 2, 2)
        sr_stage(7, oP_prev)
        drain_mm(4, 0, psA, 3, 2)
        drain_fin(4, 0, psA)
        drain_mm(4, 1, psB, 3, 2)
        drain_fin(4, 1, psB)
        for qt, n in ((5, 0), (5, 1)):
            ps = gtile()
            drain_mm(qt, n, ps, 2, 2)
            drain_mm(qt, n, ps, 3, 2)
            drain_fin(qt, n, ps)
        for qt, n in ((6, 0), (6, 1), (7, 0), (7, 1)):
            ps = gtile()
            drain_mm(qt, n, ps, 3, 3)
            drain_fin(qt, n, ps)


def get_nc():
    if "nc" not in _NC_CACHE:
        _NC_CACHE["nc"] = _build_nc()
    return _NC_CACHE["nc"]


def _swz(a, nblk, blk, width):
    """[nblk*blk, width] -> [blk, nblk*width] SBUF image (p t n)."""
    return np.ascontiguousarray(
        a.reshape(nblk, blk, width).transpose(1, 0, 2).reshape(blk, -1))


def make_in_maps(q, x, q_mask, k_mask, Wq, bq, Wk, bk, Wv, bv, Wo, bo):
    """Host-side shard/layout prep. Returns in_maps for cores 0..7."""
    q = np.asarray(q, np.float32)
    x = np.asarray(x, np.float32)
    in_maps = []
    for c in range(NCORES):
        b, hh = c // 2, c % 2
        hs = slice(VS * hh, VS * (hh + 1))
        qT = _swz(np.ascontiguousarray(q[b].T).astype(BF16), 8, 128, 1024)
        xT = _swz(np.ascontiguousarray(x[b].T).astype(BF16), 8, 128, 1024)
        wq = _swz(np.ascontiguousarray(Wq[:, hs]).astype(BF16), 8, 128, 512)
        wk = _swz(np.ascontiguousarray(Wk[:, hs]).astype(BF16), 8, 128, 512)
        wv = _swz(np.ascontiguousarray(Wv[:, hs]).astype(BF16), 8, 128, 512)
        wo = _swz(np.ascontiguousarray(Wo[hs, :]).astype(BF16), 4, 128, 1024)
        kbias = np.where(np.asarray(k_mask)[b] != 0, 0.0, NEG).astype(np.float32)
        consts = np.empty((128, 24), np.float32)
        consts[:, 0:4] = np.asarray(bq, np.float32)[hs].reshape(4, 128).T
        consts[:, 4:8] = np.asarray(bk, np.float32)[hs].reshape(4, 128).T
        consts[:, 8:16] = kbias.reshape(8, 128).T
        consts[:, 16:24] = (np.asarray(q_mask)[b].astype(np.float32)
                            .reshape(8, 128).T)
        in_maps.append({
            "inA": np.ascontiguousarray(np.concatenate([qT, wq], axis=1)),
            "inB": np.ascontiguousarray(np.concatenate([xT, wk], axis=1)),
            "inC": wv,
            "inD": wo,
            "consts": np.ascontiguousarray(consts),
        })
    return in_maps


def unswizzle_out(res_out):
    """[128, 8192] device image -> [1024, 1024] partial (fp32)."""
    a = np.asarray(res_out).astype(np.float32)
    return a.reshape(128, 8, 2, 512).transpose(1, 0, 2, 3).reshape(1024, 1024)


def kernel(q, x, q_mask, k_mask, Wq, bq, Wk, bk, Wv, bv, Wo, bo):
    from concourse import bass_utils

    q_mask = np.asarray(q_mask)
    k_mask = np.asarray(k_mask)

    nc = get_nc()
    in_maps = make_in_maps(q, x, q_mask, k_mask, Wq, bq, Wk, bk, Wv, bv, Wo, bo)
    res = bass_utils.run_bass_kernel_spmd(nc, in_maps,
                                          core_ids=list(range(NCORES)))

    out = np.empty((B, LQ, D), np.float32)
    for b in range(B):
        out[b] = (unswizzle_out(res.results[2 * b]["out"])
                  + unswizzle_out(res.results[2 * b + 1]["out"]))
    # exact host-side bias fold: attn rows sum to 1 pre-q_mask, so
    # out = (attn@V)@Wo on device and +q_mask*(bv@Wo) + bo here.
    bvwo = np.asarray(bv, np.float32) @ np.asarray(Wo, np.float32)
    bo_f = np.asarray(bo, np.float32)
    if np.any(bvwo) or np.any(bo_f):
        out += (q_mask.astype(np.float32)[:, :, None] * bvwo[None, None, :]
                + bo_f[None, None, :])
    return out


# revision 8
# speedup vs baseline: 1.5427x; 1.5427x over previous
"""Multi-head attention (B=4, L=1024, D=1024, H=16) on 8 TRN2 NeuronCores.

v3: head-split data-parallel sharding. Core c handles batch c//2 and HEADS
[8*(c%2), 8*(c%2)+8) over ALL 1024 queries (v2 split queries instead and
computed K/V projections redundantly on both cores of a batch pair). The
out-projection is computed as a PARTIAL sum over this core's 512 vd dims;
the host adds the two partials of each batch pair. Per-core matmul work
drops 8.59 -> 6.44 GFLOP (the ideal 1/8 of total).

Input DMA: all tensors are pre-swizzled on the host into exact SBUF images
[128, N] so every DMA descriptor is a multi-KB contiguous per-partition
line (v2's "(t p) n -> p t n" views produced 1-2KB descriptors and landed
at ~200 GB/s; Wv arrived at 48us and stalled the whole pipeline).

Layouts per core (hh = c%2 head half, hs = 512*hh slice of VD):
  inA [128, 12288] = qT (8 kd-blocks x 1024 q) | Wq[:, hs] (8 x 512)
  inB [128, 12288] = xT (8 kd-blocks x 1024 k) | Wk[:, hs] (8 x 512)
  inC [128, 4096]  = Wv[:, hs]  (8 kd-blocks x 512)
  inD [128, 4096]  = Wo[hs, :]  (4 j-blocks x 1024)
  consts [128, 24] = bq(4) | bk(4) | kbias(8) | q_mask(8)
  out [128, 8192]  = 16 tiles (qt, n) of [128 q, 512 d] partial O-proj

Compute (all transposed, no transposes anywhere):
  Q^T[vd, q] = Wq(lhsT) @ qT  (+bq per-partition)   4 j-blocks x 2 q-halves
  K^T[vd, k] = Wk(lhsT) @ xT  (+bk)                 4 j-blocks x 2 k-halves
  V  [k, vd] = xT(lhsT) @ Wv                        8 k-blocks
  Units u=0..7 = (qh, j): S^T[k,2,q] per head pair (row-tiled K=64 pair),
  es = exp(S/8 + kbias) (ScalarE), acc = sum_kt es (DVE),
  den = reduce_C(acc) on GPSIMD, 1/den via DVE approx_fast,
  srs = partition_broadcast(1/den) on GPSIMD (per head half),
  O^T = [V_hA|V_hB](col-tiled M=64) @ es, oTs = O^T * srs (DVE).
  out tile (qt,n) = (sum_j oTs[j][:,qr] @ Wo_j) * q_mask  (partial)
"""

import os

os.environ.setdefault("MYCRO_LOCAL_CACHE", "1")

import numpy as np
import ml_dtypes

BF16 = ml_dtypes.bfloat16

B, LQ, LK = 4, 1024, 1024
D = 1024
H, DH = 16, 64
HC = 8          # heads per core
VS = 512        # vd dims per core
QS = 512        # queries per attention unit (2 units of 512 = 1024)
NCORES = 8
NEG = -1e4

_NC_CACHE = {}


def _build_nc():
    import concourse.bacc as bacc
    import concourse.mybir as mybir
    import concourse.tile as tile

    dt = mybir.dt

    nc = bacc.Bacc(
        "TRN2",
        debug=False,
        target_bir_lowering=False,
        num_devices=NCORES,
    )

    def din(name, shape, dtype):
        return nc.dram_tensor(name, shape, dtype, kind="ExternalInput").ap()

    aps = {
        "inA": din("inA", [128, 12288], dt.bfloat16),
        "inB": din("inB", [128, 12288], dt.bfloat16),
        "inC": din("inC", [128, 4096], dt.bfloat16),
        "inD": din("inD", [128, 4096], dt.bfloat16),
        "consts": din("consts", [128, 24], dt.float32),
        "out": nc.dram_tensor("out", [128, 8192], dt.bfloat16,
                              kind="ExternalOutput").ap(),
    }

    with tile.TileContext(nc) as tc:
        _body(tc, dt, mybir, aps)

    nc.compile()
    return nc


def _body(tc, dt, mybir, aps):
    from contextlib import ExitStack
    from concourse.tile import add_dep_helper

    ALU = mybir.AluOpType
    AF = mybir.ActivationFunctionType
    AX = mybir.AxisListType
    import concourse.bass_isa as bass_isa
    nc = tc.nc
    with ExitStack() as ctx:
        const = ctx.enter_context(tc.tile_pool(name="const", bufs=1))
        espool = ctx.enter_context(tc.tile_pool(name="es", bufs=12))
        accpool = ctx.enter_context(tc.tile_pool(name="acc", bufs=3))
        scpool = ctx.enter_context(tc.tile_pool(name="sc", bufs=2))
        srspool = ctx.enter_context(tc.tile_pool(name="srs", bufs=2))
        spair = ctx.enter_context(tc.tile_pool(name="sp", bufs=2, space="PSUM"))
        oppool = ctx.enter_context(tc.tile_pool(name="op", bufs=2, space="PSUM"))
        gpool = ctx.enter_context(tc.tile_pool(name="g", bufs=2, space="PSUM"))
        opool = ctx.enter_context(tc.tile_pool(name="osb", bufs=3))

        def ctile(shape, dtype, tag):
            return const.tile(shape, dtype, tag=tag, name=tag)

        def gtile():
            return gpool.tile([128, 512], dt.float32, tag="g", name="g")

        # ---- consts ----
        cst = ctile([128, 24], dt.float32, "cst")
        nc.sync.dma_start(cst[:], aps["consts"][:, :])

        bq_c = lambda j: cst[:, j:j + 1]
        bk_c = lambda j: cst[:, 4 + j:5 + j]
        kb_c = lambda kt: cst[:, 8 + kt:9 + kt]
        qm_c = lambda qt: cst[:, 16 + qt:17 + qt]

        ones1 = ctile([1, 128], dt.bfloat16, "ones1")
        nc.vector.memset(ones1[:], 1.0)
        ones512 = ctile([1, 512], dt.bfloat16, "ones512")
        nc.vector.memset(ones512[:], 1.0)
        ones64 = ctile([1, 64], dt.bfloat16, "ones64")
        nc.vector.memset(ones64[:], 1.0)
        onescol = ctile([128, 1], dt.bfloat16, "onescol")
        nc.vector.memset(onescol[:], 1.0)
        ejunk = ctile([1, 16], dt.float32, "ejunk")
        nc.vector.memset(ejunk[:], 1.0)
        # pull the exp ACT table load off the critical path
        nc.scalar.activation(ejunk[:], ejunk[:], AF.Exp, bias=0.0, scale=1.0)

        # ---- keep-alive matmuls (bridge consts->inA landing, warm HAM) ----
        ka = gtile()
        for _ in range(10):
            nc.tensor.matmul(ka[:], ones1[:], ones512[:], start=True, stop=True)

        # ---- input loads, phase-serialized A -> B -> C -> D via direct
        # DMA->DMA deps. Each dma_start moves a contiguous [128, N] SBUF
        # image (24KB/partition descriptors). ----
        inA_sb = ctile([128, 12288], dt.bfloat16, "inA")
        inB_sb = ctile([128, 12288], dt.bfloat16, "inB")
        wv_sb = ctile([128, 4096], dt.bfloat16, "wv")
        wo_sb = ctile([128, 4096], dt.bfloat16, "wo")

        engs = [nc.sync, nc.scalar]
        ai = []
        for i, eng in ((0, nc.sync), (1, nc.scalar), (2, nc.sync)):
            c = slice(4096 * i, 4096 * (i + 1))
            ai.append(eng.dma_start(inA_sb[:, c], aps["inA"][:, c]))
        bi = []
        for i, eng in ((0, nc.sync), (1, nc.scalar), (2, nc.sync)):
            c = slice(4096 * i, 4096 * (i + 1))
            bi.append(eng.dma_start(inB_sb[:, c], aps["inB"][:, c]))
        ci = [nc.sync.dma_start(wv_sb[:], aps["inC"][:, :])]
        di = [nc.sync.dma_start(wo_sb[:], aps["inD"][:, :])]
        for nxt, prv in ((bi, ai), (ci, bi), (di, ci)):
            for n_ in nxt:
                for p_ in prv:
                    add_dep_helper(n_.ins, p_.ins, reason="dma phase order")

        # views into the flat input tiles
        def qT_v(kd, qh):
            return inA_sb[:, 1024 * kd + 512 * qh:1024 * kd + 512 * (qh + 1)]

        def wq_v(kd, j):
            return inA_sb[:, 8192 + 512 * kd + 128 * j:
                          8192 + 512 * kd + 128 * (j + 1)]

        def xT_v(kd, n):
            return inB_sb[:, 1024 * kd + 512 * n:1024 * kd + 512 * (n + 1)]

        def xT_vb(kd, t):
            return inB_sb[:, 1024 * kd + 128 * t:1024 * kd + 128 * (t + 1)]

        def wk_v(kd, j):
            return inB_sb[:, 8192 + 512 * kd + 128 * j:
                          8192 + 512 * kd + 128 * (j + 1)]

        def wv_v(kd):
            return wv_sb[:, 512 * kd:512 * (kd + 1)]

        def wo_v(j, n):
            return wo_sb[:, 1024 * j + 512 * n:1024 * j + 512 * (n + 1)]

        # ---- projections ----
        qTp = [ctile([128, 1024], dt.bfloat16, f"qTp{j}") for j in range(4)]
        kT_sb = [ctile([128, 1024], dt.bfloat16, f"kT{j}") for j in range(4)]
        v_sb = [ctile([128, 512], dt.bfloat16, f"v{t}") for t in range(8)]

        def q_proj(j, qh):
            c = slice(512 * qh, 512 * (qh + 1))
            ps = gtile()
            for kd in range(8):
                nc.tensor.matmul(ps[:], wq_v(kd, j), qT_v(kd, qh),
                                 start=(kd == 0), stop=(kd == 7))
            nc.vector.tensor_scalar_add(qTp[j][:, c], ps[:], bq_c(j))

        def k_proj(j, n):
            c = slice(512 * n, 512 * (n + 1))
            ps = gtile()
            for kd in range(8):
                nc.tensor.matmul(ps[:], wk_v(kd, j), xT_v(kd, n),
                                 start=(kd == 0), stop=(kd == 7))
            nc.vector.tensor_scalar_add(kT_sb[j][:, c], ps[:], bk_c(j))

        def v_proj(t):
            ps = gtile()
            for kd in range(8):
                nc.tensor.matmul(ps[:], xT_vb(kd, t), wv_v(kd),
                                 start=(kd == 0), stop=(kd == 7))
            nc.vector.tensor_copy(v_sb[t][:], ps[:])

        # ---- attention units: u -> (qh, j) ----
        UNITS = [(0, 0), (0, 1), (0, 2), (0, 3), (1, 0), (1, 1), (1, 2), (1, 3)]
        oTs = [ctile([128, 1024], dt.bfloat16, f"oTs{j}") for j in range(4)]
        es_tiles = {}
        acc_last = {}
        srs_of = {}

        def s_stage(u, kt):
            qh, j = UNITS[u]
            qs = slice(512 * qh, 512 * (qh + 1))
            kc = slice(128 * kt, 128 * (kt + 1))
            sp = spair.tile([128, 2, 512], dt.float32, tag="sp", name="sp")
            nc.tensor.matmul(sp[:, 0, :], kT_sb[j][0:64, kc],
                             qTp[j][0:64, qs], start=True, stop=True)
            nc.tensor.matmul(sp[:, 1, :], kT_sb[j][64:128, kc],
                             qTp[j][64:128, qs], start=True, stop=True)
            es = espool.tile([128, 2, 512], dt.bfloat16, tag="es", name="es")
            nc.scalar.activation(es[:], sp[:], AF.Exp,
                                 bias=kb_c(kt), scale=0.125)
            es_tiles[(u, kt)] = es
            if kt == 0:
                acc_last[u] = es
            else:
                a = accpool.tile([128, 2, 512], dt.bfloat16, tag="acc",
                                 name="acc")
                prev = acc_last[u]
                nc.vector.tensor_add(
                    a[:].rearrange("p h q -> p (h q)"),
                    prev[:].rearrange("p h q -> p (h q)"),
                    es[:].rearrange("p h q -> p (h q)"))
                acc_last[u] = a

        def o_stage(u, kt, oP):
            qh, j = UNITS[u]
            es = es_tiles.pop((u, kt))
            nc.tensor.matmul(oP[0:64, :], v_sb[kt][:, 128 * j:128 * j + 64],
                             es[:, 0, :], start=(kt == 0), stop=(kt == 7),
                             tile_position=(0, 0), skip_group_check=True)
            nc.tensor.matmul(oP[64:128, :],
                             v_sb[kt][:, 128 * j + 64:128 * j + 128],
                             es[:, 1, :], start=(kt == 0), stop=(kt == 7),
                             tile_position=(0, 64), skip_group_check=True)

        def den_stage(u):
            # den via two M=1 PE matmuls into partition 0 (GPSIMD
            # partition_all_reduce measured 6.7us/op on HW - unusable).
            # reciprocal_approx_fast drops the partition base of its input
            # AP, so each denominator goes to partition 0 of its own tile.
            a = acc_last.pop(u)
            dpA = gtile()
            nc.tensor.matmul(dpA[0:1, :], onescol[:], a[:, 0, :],
                             start=True, stop=True)
            dpB = gtile()
            nc.tensor.matmul(dpB[0:1, :], onescol[:], a[:, 1, :],
                             start=True, stop=True)
            sca = scpool.tile([1, 1024], dt.float32, tag="sca", name="sca")
            scb = scpool.tile([1, 1024], dt.bfloat16, tag="scb", name="scb")
            nc.vector.reciprocal_approx_fast(out=sca[:, 0:512], in_=dpA[0:1, :])
            nc.vector.reciprocal_approx_fast(out=sca[:, 512:1024],
                                             in_=dpB[0:1, :])
            nc.scalar.copy(scb[:], sca[:])  # cast on ScalarE: DVE is loaded
            srs_of[u] = scb

        def sr_stage(u, oP):
            qh, j = UNITS[u]
            qs = slice(512 * qh, 512 * (qh + 1))
            scb = srs_of.pop(u)
            sr = gtile()
            nc.tensor.matmul(sr[0:64, :], ones64[:], scb[:, 0:512],
                             start=True, stop=True)
            nc.tensor.matmul(sr[64:128, :], ones64[:], scb[:, 512:1024],
                             start=True, stop=True, tile_position=(0, 64),
                             skip_group_check=True)
            # DVE can read at most one PSUM operand: stage sr to SBUF first
            srs = srspool.tile([128, 512], dt.bfloat16, tag="srs", name="srs")
            nc.vector.tensor_copy(srs[:], sr[:])
            nc.vector.tensor_mul(oTs[j][:, qs], oP[:], srs[:])

        # ---- out-projection (partial over this core's 512 vd dims) ----
        dei = [0]

        def out_dma(ot, qt, n):
            c = slice(512 * (2 * qt + n), 512 * (2 * qt + n + 1))
            nc.sync.dma_start(aps["out"][:, c], ot[:])
            dei[0] += 1

        def out_tile(qt, n):
            c = slice(512 * n, 512 * (n + 1))
            qr = slice(128 * qt, 128 * (qt + 1))
            ps = gtile()
            for j in range(4):
                nc.tensor.matmul(ps[:], oTs[j][:, qr], wo_v(j, n),
                                 start=(j == 0), stop=(j == 3))
            ot = opool.tile([128, 512], dt.bfloat16, tag="osb", name="osb")
            nc.vector.tensor_scalar_mul(ot[:], ps[:], qm_c(qt))
            out_dma(ot, qt, n)

        stage_sbuf = {}

        def stage_partial(qt, n, upto):
            c = slice(512 * n, 512 * (n + 1))
            qr = slice(128 * qt, 128 * (qt + 1))
            ps = gtile()
            for j in range(upto):
                nc.tensor.matmul(ps[:], oTs[j][:, qr], wo_v(j, n),
                                 start=(j == 0), stop=(j == upto - 1))
            st = ctile([128, 512], dt.float32, f"stg{qt}{n}")
            nc.vector.tensor_scalar_mul(st[:], ps[:], qm_c(qt))
            stage_sbuf[(qt, n)] = (st, upto)

        def drain_mm(qt, n, ps, j, upto):
            c = slice(512 * n, 512 * (n + 1))
            qr = slice(128 * qt, 128 * (qt + 1))
            nc.tensor.matmul(ps[:], oTs[j][:, qr], wo_v(j, n),
                             start=(j == upto), stop=(j == 3))

        def drain_fin(qt, n, ps):
            st, _ = stage_sbuf[(qt, n)]
            ot = opool.tile([128, 512], dt.bfloat16, tag="osb", name="osb")
            nc.vector.scalar_tensor_tensor(
                ot[:], ps[:], qm_c(qt), st[:], op0=ALU.mult, op1=ALU.add)
            out_dma(ot, qt, n)

        # ---- schedule ----
        LOOK = 2
        s_cursor = [0]

        def advance_s(upto):
            while s_cursor[0] < min(upto, 64):
                uu, kk = divmod(s_cursor[0], 8)
                s_stage(uu, kk)
                s_cursor[0] += 1

        # per-(unit, kt2-block) PE filler emissions. Placement rules (the PE
        # queue is in-order, advance_s at block (u,6) emits next-unit S):
        # k(j',0) before block (u_{j'-1}, 6); k(j',1) before block (u_{j'}, 2);
        # v(t) before block (u0, t).
        fillers = {
            (0, 0): [lambda: v_proj(2), lambda: v_proj(3)],
            (0, 2): [lambda: v_proj(4), lambda: v_proj(5)],
            (0, 4): [lambda: k_proj(1, 0), lambda: v_proj(6), lambda: v_proj(7)],
            (0, 6): [lambda: k_proj(1, 1)],
            (1, 0): [lambda: k_proj(2, 0)],
            (1, 2): [lambda: q_proj(0, 1)],
            (1, 4): [lambda: k_proj(2, 1)],
            (2, 0): [lambda: k_proj(3, 0)],
            (2, 2): [lambda: q_proj(1, 1)],
            (2, 4): [lambda: k_proj(3, 1)],
            (3, 0): [lambda: q_proj(2, 1)],
            (3, 4): [lambda: q_proj(3, 1)],
            # qh=0 out tiles once sr(u3) has run (block (4,2))
            (4, 4): [lambda: out_tile(0, 0), lambda: out_tile(0, 1)],
            (4, 6): [lambda: out_tile(1, 0), lambda: out_tile(1, 1)],
            (5, 0): [lambda: out_tile(2, 0), lambda: out_tile(2, 1)],
            (5, 2): [lambda: out_tile(3, 0), lambda: out_tile(3, 1)],
            # qh=1 partial staging: j0,j1 after sr(u5) at (6,2); j0..j2
            # after sr(u6) at (7,2)
            (6, 4): [lambda: stage_partial(4, 0, 2),
                     lambda: stage_partial(4, 1, 2)],
            (6, 6): [lambda: stage_partial(5, 0, 2),
                     lambda: stage_partial(5, 1, 2)],
            (7, 2): [lambda: stage_partial(6, 0, 3),
                     lambda: stage_partial(6, 1, 3)],
            (7, 4): [lambda: stage_partial(7, 0, 3),
                     lambda: stage_partial(7, 1, 3)],
        }

        # prologue
        q_proj(0, 0)
        q_proj(1, 0)
        q_proj(2, 0)
        q_proj(3, 0)
        k_proj(0, 0)
        k_proj(0, 1)
        advance_s(2)
        v_proj(0)
        v_proj(1)

        oP_prev = None
        for u in range(8):
            oP = oppool.tile([128, 512], dt.float32, tag="op", name="op")
            for kt2 in range(0, 8, 2):
                if kt2 == 2 and u > 0:
                    sr_stage(u - 1, oP_prev)
                advance_s(8 * u + kt2 + 2 + LOOK)
                o_stage(u, kt2, oP)
                o_stage(u, kt2 + 1, oP)
                for f in fillers.get((u, kt2), []):
                    f()
                for f in fillers.get((u, kt2 + 1), []):
                    f()
            den_stage(u)
            oP_prev = oP

        # ---- drain ----
        # sr(7) FIRST: a gpool chain opened before it would deadlock the
        # 2-slot rotation (its release rides a PE matmul emitted later).
        sr_stage(7, oP_prev)
        for qt, n in ((4, 0), (4, 1), (5, 0), (5, 1),
                      (6, 0), (6, 1), (7, 0), (7, 1)):
            _, upto = stage_sbuf[(qt, n)]
            ps = gtile()
            for j in range(upto, 4):
                drain_mm(qt, n, ps, j, upto)
            drain_fin(qt, n, ps)


def get_nc():
    if "nc" not in _NC_CACHE:
        _NC_CACHE["nc"] = _build_nc()
    return _NC_CACHE["nc"]


def _swz(a, nblk, blk, width):
    """[nblk*blk, width] -> [blk, nblk*width] SBUF image (p t n)."""
    return np.ascontiguousarray(
        a.reshape(nblk, blk, width).transpose(1, 0, 2).reshape(blk, -1))


def make_in_maps(q, x, q_mask, k_mask, Wq, bq, Wk, bk, Wv, bv, Wo, bo):
    """Host-side shard/layout prep. Returns in_maps for cores 0..7."""
    q = np.asarray(q, np.float32)
    x = np.asarray(x, np.float32)
    in_maps = []
    for c in range(NCORES):
        b, hh = c // 2, c % 2
        hs = slice(VS * hh, VS * (hh + 1))
        qT = _swz(np.ascontiguousarray(q[b].T).astype(BF16), 8, 128, 1024)
        xT = _swz(np.ascontiguousarray(x[b].T).astype(BF16), 8, 128, 1024)
        wq = _swz(np.ascontiguousarray(Wq[:, hs]).astype(BF16), 8, 128, 512)
        wk = _swz(np.ascontiguousarray(Wk[:, hs]).astype(BF16), 8, 128, 512)
        wv = _swz(np.ascontiguousarray(Wv[:, hs]).astype(BF16), 8, 128, 512)
        wo = _swz(np.ascontiguousarray(Wo[hs, :]).astype(BF16), 4, 128, 1024)
        kbias = np.where(np.asarray(k_mask)[b] != 0, 0.0, NEG).astype(np.float32)
        consts = np.empty((128, 24), np.float32)
        consts[:, 0:4] = np.asarray(bq, np.float32)[hs].reshape(4, 128).T
        consts[:, 4:8] = np.asarray(bk, np.float32)[hs].reshape(4, 128).T
        consts[:, 8:16] = kbias.reshape(8, 128).T
        consts[:, 16:24] = (np.asarray(q_mask)[b].astype(np.float32)
                            .reshape(8, 128).T)
        in_maps.append({
            "inA": np.ascontiguousarray(np.concatenate([qT, wq], axis=1)),
            "inB": np.ascontiguousarray(np.concatenate([xT, wk], axis=1)),
            "inC": wv,
            "inD": wo,
            "consts": np.ascontiguousarray(consts),
        })
    return in_maps


def unswizzle_out(res_out):
    """[128, 8192] device image -> [1024, 1024] partial (fp32)."""
    a = np.asarray(res_out).astype(np.float32)
    return a.reshape(128, 8, 2, 512).transpose(1, 0, 2, 3).reshape(1024, 1024)


def kernel(q, x, q_mask, k_mask, Wq, bq, Wk, bk, Wv, bv, Wo, bo):
    from concourse import bass_utils

    q_mask = np.asarray(q_mask)
    k_mask = np.asarray(k_mask)

    nc = get_nc()
    in_maps = make_in_maps(q, x, q_mask, k_mask, Wq, bq, Wk, bk, Wv, bv, Wo, bo)
    res = bass_utils.run_bass_kernel_spmd(nc, in_maps,
                                          core_ids=list(range(NCORES)))

    out = np.empty((B, LQ, D), np.float32)
    for b in range(B):
        out[b] = (unswizzle_out(res.results[2 * b]["out"])
                  + unswizzle_out(res.results[2 * b + 1]["out"]))
    # exact host-side bias fold: attn rows sum to 1 pre-q_mask, so
    # out = (attn@V)@Wo on device and +q_mask*(bv@Wo) + bo here.
    bvwo = np.asarray(bv, np.float32) @ np.asarray(Wo, np.float32)
    bo_f = np.asarray(bo, np.float32)
    if np.any(bvwo) or np.any(bo_f):
        out += (q_mask.astype(np.float32)[:, :, None] * bvwo[None, None, :]
                + bo_f[None, None, :])
    return out


# revision 15
# speedup vs baseline: 1.5541x; 1.0074x over previous
"""Multi-head attention (B=4, L=1024, D=1024, H=16) on 8 TRN2 NeuronCores.

v3: head-split data-parallel sharding. Core c handles batch c//2 and HEADS
[8*(c%2), 8*(c%2)+8) over ALL 1024 queries (v2 split queries instead and
computed K/V projections redundantly on both cores of a batch pair). The
out-projection is computed as a PARTIAL sum over this core's 512 vd dims;
the host adds the two partials of each batch pair. Per-core matmul work
drops 8.59 -> 6.44 GFLOP (the ideal 1/8 of total).

Input DMA: all tensors are pre-swizzled on the host into exact SBUF images
[128, N] so every DMA descriptor is a multi-KB contiguous per-partition
line (v2's "(t p) n -> p t n" views produced 1-2KB descriptors and landed
at ~200 GB/s; Wv arrived at 48us and stalled the whole pipeline).

Layouts per core (hh = c%2 head half, hs = 512*hh slice of VD):
  inA [128, 12288] = qT (8 kd-blocks x 1024 q) | Wq[:, hs] (8 x 512)
  inB [128, 12288] = xT (8 kd-blocks x 1024 k) | Wk[:, hs] (8 x 512)
  inC [128, 4096]  = Wv[:, hs]  (8 kd-blocks x 512)
  inD [128, 4096]  = Wo[hs, :]  (4 j-blocks x 1024)
  consts [128, 24] = bq(4) | bk(4) | kbias(8) | q_mask(8)
  out [128, 8192]  = 16 tiles (qt, n) of [128 q, 512 d] partial O-proj

Compute (all transposed, no transposes anywhere):
  Q^T[vd, q] = Wq(lhsT) @ qT  (+bq per-partition)   4 j-blocks x 2 q-halves
  K^T[vd, k] = Wk(lhsT) @ xT  (+bk)                 4 j-blocks x 2 k-halves
  V  [k, vd] = xT(lhsT) @ Wv                        8 k-blocks
  Units u=0..7 = (qh, j): S^T[k,2,q] per head pair (row-tiled K=64 pair),
  es = exp(S/8 + kbias) (ScalarE), acc = sum_kt es (DVE),
  den = reduce_C(acc) on GPSIMD, 1/den via DVE approx_fast,
  srs = partition_broadcast(1/den) on GPSIMD (per head half),
  O^T = [V_hA|V_hB](col-tiled M=64) @ es, oTs = O^T * srs (DVE).
  out tile (qt,n) = (sum_j oTs[j][:,qr] @ Wo_j) * q_mask  (partial)
"""

import os

os.environ.setdefault("MYCRO_LOCAL_CACHE", "1")

import numpy as np
import ml_dtypes

BF16 = ml_dtypes.bfloat16

B, LQ, LK = 4, 1024, 1024
D = 1024
H, DH = 16, 64
HC = 8          # heads per core
VS = 512        # vd dims per core
QS = 512        # queries per attention unit (2 units of 512 = 1024)
NCORES = 8
NEG = -1e4

_NC_CACHE = {}


def _build_nc():
    import concourse.bacc as bacc
    import concourse.mybir as mybir
    import concourse.tile as tile

    dt = mybir.dt

    nc = bacc.Bacc(
        "TRN2",
        debug=False,
        target_bir_lowering=False,
        num_devices=NCORES,
    )

    def din(name, shape, dtype):
        return nc.dram_tensor(name, shape, dtype, kind="ExternalInput").ap()

    aps = {
        "inA": din("inA", [128, 12288], dt.bfloat16),
        "inB": din("inB", [128, 12288], dt.bfloat16),
        "inC": din("inC", [128, 4096], dt.bfloat16),
        "inD": din("inD", [128, 4096], dt.bfloat16),
        "consts": din("consts", [128, 24], dt.float32),
        "out": nc.dram_tensor("out", [128, 8192], dt.bfloat16,
                              kind="ExternalOutput").ap(),
    }

    with tile.TileContext(nc) as tc:
        _body(tc, dt, mybir, aps)

    nc.compile()
    return nc


def _body(tc, dt, mybir, aps):
    from contextlib import ExitStack
    from concourse.tile import add_dep_helper

    ALU = mybir.AluOpType
    AF = mybir.ActivationFunctionType
    AX = mybir.AxisListType
    import concourse.bass_isa as bass_isa
    nc = tc.nc
    with ExitStack() as ctx:
        const = ctx.enter_context(tc.tile_pool(name="const", bufs=1))
        espool = ctx.enter_context(tc.tile_pool(name="es", bufs=12))
        accpool = ctx.enter_context(tc.tile_pool(name="acc", bufs=3))
        scpool = ctx.enter_context(tc.tile_pool(name="sc", bufs=2))
        srspool = ctx.enter_context(tc.tile_pool(name="srs", bufs=2))
        spair = ctx.enter_context(tc.tile_pool(name="sp", bufs=2, space="PSUM"))
        oppool = ctx.enter_context(tc.tile_pool(name="op", bufs=2, space="PSUM"))
        gpool = ctx.enter_context(tc.tile_pool(name="g", bufs=2, space="PSUM"))
        opool = ctx.enter_context(tc.tile_pool(name="osb", bufs=3))

        def ctile(shape, dtype, tag):
            return const.tile(shape, dtype, tag=tag, name=tag)

        def gtile():
            return gpool.tile([128, 512], dt.float32, tag="g", name="g")

        # ---- consts ----
        cst = ctile([128, 24], dt.float32, "cst")
        nc.sync.dma_start(cst[:], aps["consts"][:, :])

        bq_c = lambda j: cst[:, j:j + 1]
        bk_c = lambda j: cst[:, 4 + j:5 + j]
        kb_c = lambda kt: cst[:, 8 + kt:9 + kt]
        qm_c = lambda qt: cst[:, 16 + qt:17 + qt]

        ones64 = ctile([1, 64], dt.bfloat16, "ones64")
        nc.vector.memset(ones64[:], 1.0)
        onescol = ctile([128, 1], dt.bfloat16, "onescol")
        nc.vector.memset(onescol[:], 1.0)
        ejunk = ctile([1, 16], dt.float32, "ejunk")
        nc.vector.memset(ejunk[:], 1.0)
        # pull the exp ACT table load off the critical path
        nc.scalar.activation(ejunk[:], ejunk[:], AF.Exp, bias=0.0, scale=1.0)
        # keep-alive operand: K must be 128 (full rows) or the PE HAM
        # activity monitor never registers busy and the clock stays 1.2 GHz
        kaw = ctile([128, 512], dt.bfloat16, "kaw")
        nc.vector.memset(kaw[:], 0.0)

        def ka(n):
            kat = gtile()
            for _ in range(n):
                nc.tensor.matmul(kat[:], kaw[:, 0:128], kaw[:],
                                 start=True, stop=True)

        # ---- input loads. Images are ordered so the minimal bytes for each
        # pipeline step land first, phases serialized via DMA->DMA deps:
        # A1 qT_qh0 -> A2 Wq  (q_proj qh0 chains)
        # B1 xT_n0 -> B2 Wk   (k_proj n0 halves)
        # C Wv -> B3 xT_n1 -> A3 qT_qh1 -> D Wo
        # Each dma_start moves a contiguous [128, 4096] SBUF image. ----
        inA_sb = ctile([128, 12288], dt.bfloat16, "inA")
        inB_sb = ctile([128, 12288], dt.bfloat16, "inB")
        wv_sb = ctile([128, 4096], dt.bfloat16, "wv")
        wo_sb = ctile([128, 4096], dt.bfloat16, "wo")

        def _dma(eng, tl, apn, i):
            c = slice(4096 * i, 4096 * (i + 1))
            return eng.dma_start(tl[:, c], aps[apn][:, c])

        phases = [
            [_dma(nc.sync, inA_sb, "inA", 0), _dma(nc.scalar, inA_sb, "inA", 1)],
            [_dma(nc.sync, inB_sb, "inB", 0), _dma(nc.scalar, inB_sb, "inB", 1)],
            [nc.sync.dma_start(wv_sb[:], aps["inC"][:, :])],
            [_dma(nc.scalar, inB_sb, "inB", 2)],
            [_dma(nc.sync, inA_sb, "inA", 2)],
            [nc.scalar.dma_start(wo_sb[:], aps["inD"][:, :])],
        ]
        for nxt, prv in zip(phases[1:], phases):
            for n_ in nxt:
                for p_ in prv:
                    add_dep_helper(n_.ins, p_.ins, reason="dma phase order")

        # views into the flat input tiles
        # inA = [qT_qh0 (8kd x 512) | Wq (8kd x 512) | qT_qh1 (8kd x 512)]
        # inB = [xT_n0 (8kd x 512) | Wk (8kd x 512) | xT_n1 (8kd x 512)]
        def qT_v(kd, qh):
            return inA_sb[:, 8192 * qh + 512 * kd:8192 * qh + 512 * (kd + 1)]

        def wq_v(kd, j):
            return inA_sb[:, 4096 + 512 * kd + 128 * j:
                          4096 + 512 * kd + 128 * (j + 1)]

        def xT_v(kd, n):
            return inB_sb[:, 8192 * n + 512 * kd:8192 * n + 512 * (kd + 1)]

        def xT_vb(kd, t):
            base = 8192 * (t // 4) + 512 * kd + 128 * (t % 4)
            return inB_sb[:, base:base + 128]

        def wk_v(kd, j):
            return inB_sb[:, 4096 + 512 * kd + 128 * j:
                          4096 + 512 * kd + 128 * (j + 1)]

        def wv_v(kd):
            return wv_sb[:, 512 * kd:512 * (kd + 1)]

        def wo_v(j, n):
            return wo_sb[:, 1024 * j + 512 * n:1024 * j + 512 * (n + 1)]

        # ---- projections ----
        qTp = [ctile([128, 1024], dt.bfloat16, f"qTp{j}") for j in range(4)]
        kT_sb = [ctile([128, 1024], dt.bfloat16, f"kT{j}") for j in range(4)]
        v_sb = [ctile([128, 512], dt.bfloat16, f"v{t}") for t in range(8)]

        def q_proj(j, qh):
            c = slice(512 * qh, 512 * (qh + 1))
            ps = gtile()
            for kd in range(8):
                nc.tensor.matmul(ps[:], wq_v(kd, j), qT_v(kd, qh),
                                 start=(kd == 0), stop=(kd == 7))
            nc.vector.tensor_scalar_add(qTp[j][:, c], ps[:], bq_c(j))

        def k_proj(j, n):
            c = slice(512 * n, 512 * (n + 1))
            ps = gtile()
            for kd in range(8):
                nc.tensor.matmul(ps[:], wk_v(kd, j), xT_v(kd, n),
                                 start=(kd == 0), stop=(kd == 7))
            nc.vector.tensor_scalar_add(kT_sb[j][:, c], ps[:], bk_c(j))

        def v_proj(t):
            ps = gtile()
            for kd in range(8):
                nc.tensor.matmul(ps[:], xT_vb(kd, t), wv_v(kd),
                                 start=(kd == 0), stop=(kd == 7))
            nc.vector.tensor_copy(v_sb[t][:], ps[:])

        # ---- attention units: u -> (qh, j) ----
        UNITS = [(0, 0), (0, 1), (0, 2), (0, 3), (1, 0), (1, 1), (1, 2), (1, 3)]
        oTs = [ctile([128, 1024], dt.bfloat16, f"oTs{j}") for j in range(4)]
        es_tiles = {}
        acc_last = {}
        srs_of = {}

        def s_stage(u, kt):
            qh, j = UNITS[u]
            qs = slice(512 * qh, 512 * (qh + 1))
            kc = slice(128 * kt, 128 * (kt + 1))
            sp = spair.tile([128, 2, 512], dt.float32, tag="sp", name="sp")
            nc.tensor.matmul(sp[:, 0, :], kT_sb[j][0:64, kc],
                             qTp[j][0:64, qs], start=True, stop=True)
            nc.tensor.matmul(sp[:, 1, :], kT_sb[j][64:128, kc],
                             qTp[j][64:128, qs], start=True, stop=True)
            es = espool.tile([128, 2, 512], dt.bfloat16, tag="es", name="es")
            nc.scalar.activation(es[:], sp[:], AF.Exp,
                                 bias=kb_c(kt), scale=0.125)
            es_tiles[(u, kt)] = es
            if kt == 0:
                acc_last[u] = es
            else:
                a = accpool.tile([128, 2, 512], dt.bfloat16, tag="acc",
                                 name="acc")
                prev = acc_last[u]
                nc.vector.tensor_add(
                    a[:].rearrange("p h q -> p (h q)"),
                    prev[:].rearrange("p h q -> p (h q)"),
                    es[:].rearrange("p h q -> p (h q)"))
                acc_last[u] = a

        def o_stage(u, kt, oP):
            qh, j = UNITS[u]
            es = es_tiles.pop((u, kt))
            nc.tensor.matmul(oP[0:64, :], v_sb[kt][:, 128 * j:128 * j + 64],
                             es[:, 0, :], start=(kt == 0), stop=(kt == 7),
                             tile_position=(0, 0), skip_group_check=True)
            nc.tensor.matmul(oP[64:128, :],
                             v_sb[kt][:, 128 * j + 64:128 * j + 128],
                             es[:, 1, :], start=(kt == 0), stop=(kt == 7),
                             tile_position=(0, 64), skip_group_check=True)

        def den_stage(u):
            # den via two M=1 PE matmuls into partition 0 (GPSIMD
            # partition_all_reduce measured 6.7us/op on HW - unusable).
            # reciprocal_approx_fast drops the partition base of its input
            # AP, so each denominator goes to partition 0 of its own tile.
            a = acc_last.pop(u)
            dpA = gtile()
            nc.tensor.matmul(dpA[0:1, :], onescol[:], a[:, 0, :],
                             start=True, stop=True)
            dpB = gtile()
            nc.tensor.matmul(dpB[0:1, :], onescol[:], a[:, 1, :],
                             start=True, stop=True)
            sca = scpool.tile([1, 1024], dt.float32, tag="sca", name="sca")
            scb = scpool.tile([1, 1024], dt.bfloat16, tag="scb", name="scb")
            nc.vector.reciprocal_approx_fast(out=sca[:, 0:512], in_=dpA[0:1, :])
            nc.vector.reciprocal_approx_fast(out=sca[:, 512:1024],
                                             in_=dpB[0:1, :])
            nc.scalar.copy(scb[:], sca[:])  # cast on ScalarE: DVE is loaded
            srs_of[u] = scb

        def sr_stage(u, oP):
            qh, j = UNITS[u]
            qs = slice(512 * qh, 512 * (qh + 1))
            scb = srs_of.pop(u)
            sr = gtile()
            nc.tensor.matmul(sr[0:64, :], ones64[:], scb[:, 0:512],
                             start=True, stop=True)
            nc.tensor.matmul(sr[64:128, :], ones64[:], scb[:, 512:1024],
                             start=True, stop=True, tile_position=(0, 64),
                             skip_group_check=True)
            # DVE can read at most one PSUM operand: stage sr to SBUF first
            srs = srspool.tile([128, 512], dt.bfloat16, tag="srs", name="srs")
            nc.vector.tensor_copy(srs[:], sr[:])
            nc.vector.tensor_mul(oTs[j][:, qs], oP[:], srs[:])

        # ---- out-projection (partial over this core's 512 vd dims) ----
        dei = [0]

        def out_dma(ot, qt, n):
            c = slice(512 * (2 * qt + n), 512 * (2 * qt + n + 1))
            nc.sync.dma_start(aps["out"][:, c], ot[:])
            dei[0] += 1

        def out_tile(qt, n):
            c = slice(512 * n, 512 * (n + 1))
            qr = slice(128 * qt, 128 * (qt + 1))
            ps = gtile()
            for j in range(4):
                nc.tensor.matmul(ps[:], oTs[j][:, qr], wo_v(j, n),
                                 start=(j == 0), stop=(j == 3))
            ot = opool.tile([128, 512], dt.bfloat16, tag="osb", name="osb")
            nc.vector.tensor_scalar_mul(ot[:], ps[:], qm_c(qt))
            out_dma(ot, qt, n)

        stage_sbuf = {}

        def stage_partial(qt, n, upto):
            c = slice(512 * n, 512 * (n + 1))
            qr = slice(128 * qt, 128 * (qt + 1))
            ps = gtile()
            for j in range(upto):
                nc.tensor.matmul(ps[:], oTs[j][:, qr], wo_v(j, n),
                                 start=(j == 0), stop=(j == upto - 1))
            st = ctile([128, 512], dt.float32, f"stg{qt}{n}")
            nc.vector.tensor_scalar_mul(st[:], ps[:], qm_c(qt))
            stage_sbuf[(qt, n)] = (st, upto)

        def drain_mm(qt, n, ps, j, upto):
            c = slice(512 * n, 512 * (n + 1))
            qr = slice(128 * qt, 128 * (qt + 1))
            nc.tensor.matmul(ps[:], oTs[j][:, qr], wo_v(j, n),
                             start=(j == upto), stop=(j == 3))

        def drain_fin(qt, n, ps):
            st, _ = stage_sbuf[(qt, n)]
            ot = opool.tile([128, 512], dt.bfloat16, tag="osb", name="osb")
            nc.vector.scalar_tensor_tensor(
                ot[:], ps[:], qm_c(qt), st[:], op0=ALU.mult, op1=ALU.add)
            out_dma(ot, qt, n)

        # ---- schedule ----
        LOOK = 2
        s_cursor = [0]

        def advance_s(upto):
            while s_cursor[0] < min(upto, 64):
                uu, kk = divmod(s_cursor[0], 8)
                s_stage(uu, kk)
                s_cursor[0] += 1

        # per-(unit, kt2-block) PE filler emissions. Placement rules (the PE
        # queue is in-order, advance_s at block (u,6) emits next-unit S):
        # k(j',0) before block (u_{j'-1}, 6); k(j',1) before block (u_{j'}, 2);
        # v(t) before block (u0, t); k(0,1) before block (0,2) [S(u0,4)].
        fillers = {
            (0, 0): [lambda: v_proj(2), lambda: v_proj(3), lambda: k_proj(0, 1)],
            (0, 2): [lambda: v_proj(4), lambda: v_proj(5), lambda: k_proj(2, 0)],
            (0, 4): [lambda: v_proj(6), lambda: v_proj(7), lambda: k_proj(1, 1)],
            (0, 6): [lambda: k_proj(3, 0)],
            (1, 0): [lambda: k_proj(2, 1)],
            (1, 2): [lambda: q_proj(0, 1)],
            (1, 4): [lambda: k_proj(3, 1)],
            (2, 0): [lambda: q_proj(1, 1)],
            (2, 4): [lambda: q_proj(2, 1)],
            (3, 0): [lambda: q_proj(3, 1)],
            # qh=0 out tiles once sr(u3) has run (block (4,2))
            (4, 4): [lambda: out_tile(0, 0), lambda: out_tile(0, 1)],
            (4, 6): [lambda: out_tile(1, 0), lambda: out_tile(1, 1)],
            (5, 0): [lambda: out_tile(2, 0), lambda: out_tile(2, 1)],
            (5, 2): [lambda: out_tile(3, 0), lambda: out_tile(3, 1)],
            # qh=1 partial staging: j0,j1 after sr(u5) at (6,2); j0..j2
            # after sr(u6) at (7,2)
            (6, 4): [lambda: stage_partial(4, 0, 2),
                     lambda: stage_partial(4, 1, 2)],
            (6, 6): [lambda: stage_partial(5, 0, 2),
                     lambda: stage_partial(5, 1, 2)],
            (7, 2): [lambda: stage_partial(6, 0, 3),
                     lambda: stage_partial(6, 1, 3)],
            (7, 4): [lambda: stage_partial(7, 0, 3),
                     lambda: stage_partial(7, 1, 3)],
        }

        # prologue. ka() bridges DMA waits so HAM stays warm: consts->A2
        # (~14us) then the A2->B2 hole (~9us of DMA, only 7us of Q work).
        ka(16)
        q_proj(0, 0)
        q_proj(1, 0)
        q_proj(2, 0)
        q_proj(3, 0)
        ka(12)
        k_proj(0, 0)
        advance_s(2)
        k_proj(1, 0)
        v_proj(0)
        v_proj(1)

        oP_prev = None
        for u in range(8):
            oP = oppool.tile([128, 512], dt.float32, tag="op", name="op")
            for kt2 in range(0, 8, 2):
                if kt2 == 2 and u > 0:
                    sr_stage(u - 1, oP_prev)
                advance_s(8 * u + kt2 + 2 + LOOK)
                o_stage(u, kt2, oP)
                o_stage(u, kt2 + 1, oP)
                for f in fillers.get((u, kt2), []):
                    f()
                for f in fillers.get((u, kt2 + 1), []):
                    f()
            den_stage(u)
            oP_prev = oP

        # ---- drain ----
        # sr(7) FIRST: a gpool chain opened before it would deadlock the
        # 2-slot rotation (its release rides a PE matmul emitted later).
        sr_stage(7, oP_prev)
        for qt, n in ((4, 0), (4, 1), (5, 0), (5, 1),
                      (6, 0), (6, 1), (7, 0), (7, 1)):
            _, upto = stage_sbuf[(qt, n)]
            ps = gtile()
            for j in range(upto, 4):
                drain_mm(qt, n, ps, j, upto)
            drain_fin(qt, n, ps)


def get_nc():
    if "nc" not in _NC_CACHE:
        _NC_CACHE["nc"] = _build_nc()
    return _NC_CACHE["nc"]


def _swz(a, nblk, blk, width):
    """[nblk*blk, width] -> [blk, nblk*width] SBUF image (p t n)."""
    return np.ascontiguousarray(
        a.reshape(nblk, blk, width).transpose(1, 0, 2).reshape(blk, -1))


def make_in_maps(q, x, q_mask, k_mask, Wq, bq, Wk, bk, Wv, bv, Wo, bo):
    """Host-side shard/layout prep. Returns in_maps for cores 0..7."""
    q = np.asarray(q, np.float32)
    x = np.asarray(x, np.float32)
    in_maps = []
    for c in range(NCORES):
        b, hh = c // 2, c % 2
        hs = slice(VS * hh, VS * (hh + 1))
        # halves by query (qT) / key (xT) position: image = [h0 | W | h1]
        qT0 = _swz(np.ascontiguousarray(q[b][0:512, :].T).astype(BF16),
                   8, 128, 512)
        qT1 = _swz(np.ascontiguousarray(q[b][512:1024, :].T).astype(BF16),
                   8, 128, 512)
        xT0 = _swz(np.ascontiguousarray(x[b][0:512, :].T).astype(BF16),
                   8, 128, 512)
        xT1 = _swz(np.ascontiguousarray(x[b][512:1024, :].T).astype(BF16),
                   8, 128, 512)
        wq = _swz(np.ascontiguousarray(Wq[:, hs]).astype(BF16), 8, 128, 512)
        wk = _swz(np.ascontiguousarray(Wk[:, hs]).astype(BF16), 8, 128, 512)
        wv = _swz(np.ascontiguousarray(Wv[:, hs]).astype(BF16), 8, 128, 512)
        wo = _swz(np.ascontiguousarray(Wo[hs, :]).astype(BF16), 4, 128, 1024)
        kbias = np.where(np.asarray(k_mask)[b] != 0, 0.0, NEG).astype(np.float32)
        consts = np.empty((128, 24), np.float32)
        consts[:, 0:4] = np.asarray(bq, np.float32)[hs].reshape(4, 128).T
        consts[:, 4:8] = np.asarray(bk, np.float32)[hs].reshape(4, 128).T
        consts[:, 8:16] = kbias.reshape(8, 128).T
        consts[:, 16:24] = (np.asarray(q_mask)[b].astype(np.float32)
                            .reshape(8, 128).T)
        in_maps.append({
            "inA": np.ascontiguousarray(np.concatenate([qT0, wq, qT1], axis=1)),
            "inB": np.ascontiguousarray(np.concatenate([xT0, wk, xT1], axis=1)),
            "inC": wv,
            "inD": wo,
            "consts": np.ascontiguousarray(consts),
        })
    return in_maps


def unswizzle_out(res_out):
    """[128, 8192] device image -> [1024, 1024] partial (fp32)."""
    a = np.asarray(res_out).astype(np.float32)
    return a.reshape(128, 8, 2, 512).transpose(1, 0, 2, 3).reshape(1024, 1024)


def kernel(q, x, q_mask, k_mask, Wq, bq, Wk, bk, Wv, bv, Wo, bo):
    from concourse import bass_utils

    q_mask = np.asarray(q_mask)
    k_mask = np.asarray(k_mask)

    nc = get_nc()
    in_maps = make_in_maps(q, x, q_mask, k_mask, Wq, bq, Wk, bk, Wv, bv, Wo, bo)
    res = bass_utils.run_bass_kernel_spmd(nc, in_maps,
                                          core_ids=list(range(NCORES)))

    out = np.empty((B, LQ, D), np.float32)
    for b in range(B):
        out[b] = (unswizzle_out(res.results[2 * b]["out"])
                  + unswizzle_out(res.results[2 * b + 1]["out"]))
    # exact host-side bias fold: attn rows sum to 1 pre-q_mask, so
    # out = (attn@V)@Wo on device and +q_mask*(bv@Wo) + bo here.
    bvwo = np.asarray(bv, np.float32) @ np.asarray(Wo, np.float32)
    bo_f = np.asarray(bo, np.float32)
    if np.any(bvwo) or np.any(bo_f):
        out += (q_mask.astype(np.float32)[:, :, None] * bvwo[None, None, :]
                + bo_f[None, None, :])
    return out
